# revision 10
# baseline (speedup 1.0000x reference)
"""Trainium2 Bass kernel for nn_Caption (LSTM caption decoder).

Distribution: pure data-parallel over batch (128 -> 8 cores x 16), no
collectives. Per core: x0 projection GEMM, embedding gather (device),
input-gate GEMM, 40-step LSTM recurrence, vocab GEMM [640,512]@[512,10000].

Layout strategy: all GEMM operands bf16 (fp32 PSUM accumulation); weights
host-transposed so the contraction dim lands on partitions; outputs
produced in T-layout (feature on partitions) so biases fuse into ACT
copies as per-partition bias. LSTM runs B-layout (batch on partitions)
with per-step h transposed via PE into hiddensT, which is consumed
directly by the vocab GEMM. xg is injected into the gates PSUM via
identity matmuls (t-blocks padded to 32 partitions for alignment).
"""
import sys

sys.path.insert(0, "/opt/trn_rl_repo")

import numpy as np
import ml_dtypes

import concourse.bass as bass
import concourse.tile as tile
from concourse import bacc, mybir
from concourse.bass_utils import run_bass_kernel_spmd
from concourse.masks import make_identity

BF = mybir.dt.bfloat16
F32 = mybir.dt.float32
I32 = mybir.dt.int32
bfnp = ml_dtypes.bfloat16

B, F, E, H, V, T = 128, 1536, 512, 512, 10000, 40
NCORES = 8
BC = B // NCORES          # 16 batch rows per core
TB = 32                   # padded t-block width (partition alignment)
NTB = T * TB              # 1280 padded (t,b) columns
NB = T * BC               # 640 real (t,b) columns
G4 = 4 * H                # 2048 gate dims, order [i, f, o, g]
VP = ((V + 127) // 128) * 128  # 10112 padded vocab
NVT = VP // 128           # 79 vocab tiles

_CACHE = {}


def _build():
    if "nc" in _CACHE:
        return _CACHE["nc"]
    nc = bacc.Bacc("TRN2", target_bir_lowering=False, debug=False,
                   num_devices=NCORES)

    featT_d = nc.dram_tensor("featT", [F, BC], BF, kind="ExternalInput")
    idx_d = nc.dram_tensor("idx", [NTB, 1], I32, kind="ExternalInput")
    emb_d = nc.dram_tensor("embt", [V, E], BF, kind="ExternalInput")
    WinT_d = nc.dram_tensor("WinT", [F, E], BF, kind="ExternalInput")
    WihT_d = nc.dram_tensor("WihT", [E, G4], BF, kind="ExternalInput")
    WhhT_d = nc.dram_tensor("WhhT", [H, G4], BF, kind="ExternalInput")
    bcomb_d = nc.dram_tensor("bcomb", [G4], F32, kind="ExternalInput")
    bin_d = nc.dram_tensor("bin", [E], F32, kind="ExternalInput")
    bout_d = nc.dram_tensor("bout", [VP], F32, kind="ExternalInput")
    WoutT_d = nc.dram_tensor("WoutT", [H, VP], BF, kind="ExternalInput")
    out_d = nc.dram_tensor("out_lt", [VP, NB], F32, kind="ExternalOutput")

    with tile.TileContext(nc) as tc:
        with (
            tc.tile_pool(name="consts", bufs=1) as consts,
            tc.tile_pool(name="big", bufs=1) as big,
            tc.tile_pool(name="state", bufs=2) as state,
            tc.tile_pool(name="work", bufs=3) as work,
        ):
            # ---- constants ----
            identf = consts.tile([128, 128], F32)
            make_identity(nc, identf[:])
            identb = consts.tile([128, 128], BF)
            nc.vector.tensor_copy(identb[:], identf[:])

            WihT_sb = big.tile([128, 4, G4], BF, tag="wih")
            nc.sync.dma_start(
                WihT_sb[:], WihT_d.ap().rearrange("(k p) n -> p k n", p=128))
            WhhT_sb = big.tile([128, 4, G4], BF, tag="whh")
            nc.sync.dma_start(
                WhhT_sb[:], WhhT_d.ap().rearrange("(k p) n -> p k n", p=128))
            WinT_sb = big.tile([128, 12, E], BF, tag="win")
            nc.sync.dma_start(
                WinT_sb[:], WinT_d.ap().rearrange("(k p) n -> p k n", p=128))
            featT_sb = consts.tile([128, 12, BC], BF)
            nc.sync.dma_start(
                featT_sb[:], featT_d.ap().rearrange("(k p) b -> p k b", p=128))
            bias_bc = big.tile([128, G4], F32, tag="biasbc")
            nc.sync.dma_start(
                bias_bc[:],
                bass.AP(tensor=bcomb_d, offset=0, ap=[[0, 128], [1, G4]]))
            bin_sb = consts.tile([128, 4], F32)
            nc.sync.dma_start(
                bin_sb[:], bin_d.ap().rearrange("(k p) -> p k", p=128))
            bout_sb = consts.tile([128, NVT], F32)
            nc.sync.dma_start(
                bout_sb[:], bout_d.ap().rearrange("(k p) -> p k", p=128))
            idx_sb = consts.tile([128, 10, 1], I32)
            nc.sync.dma_start(
                idx_sb[:], idx_d.ap().rearrange("(j p) o -> p j o", p=128))

            # ---- embedding gather -> seqT (transposed via PE) ----
            seqT = big.tile([128, 4, NTB], BF, tag="seqT")
            with tc.tile_pool(name="psA", bufs=3, space="PSUM") as psA:
                for j in range(10):
                    gt = work.tile([128, E], BF, tag="gather")
                    nc.gpsimd.indirect_dma_start(
                        out=gt[:], out_offset=None, in_=emb_d.ap(),
                        in_offset=bass.IndirectOffsetOnAxis(
                            ap=idx_sb[:, j, :], axis=0))
                    for e in range(4):
                        pst = psA.tile([128, 128], BF, space="PSUM", tag="tr")
                        nc.tensor.transpose(
                            pst[:], gt[:, e * 128:(e + 1) * 128], identb[:])
                        nc.scalar.copy(
                            seqT[:, e, j * 128:(j + 1) * 128], pst[:])

                # ---- x0T = W_inT.T @ featT + b_in -> seqT[:, :, 0:BC] ----
                for m in range(4):
                    ps = psA.tile([128, BC], F32, space="PSUM", tag="x0")
                    for k in range(12):
                        nc.tensor.matmul(
                            ps[:], lhsT=WinT_sb[:, k, m * 128:(m + 1) * 128],
                            rhs=featT_sb[:, k, :],
                            start=(k == 0), stop=(k == 11))
                    nc.scalar.activation(
                        seqT[:, m, 0:BC], ps[:],
                        mybir.ActivationFunctionType.Identity,
                        bias=bin_sb[:, m:m + 1])

            # ---- xg = seqT.T @ W_ihT + bias (bf16, B-layout, padded) ----
            xg = big.tile([128, 10, G4], BF, tag="xg")
            with tc.tile_pool(name="psB", bufs=2, space="PSUM") as psB:
                for mt in range(10):
                    ps = psB.tile([128, G4], F32, space="PSUM", tag="xgps")
                    for n in range(4):
                        for k in range(4):
                            nc.tensor.matmul(
                                ps[:, n * 512:(n + 1) * 512],
                                lhsT=seqT[:, k, mt * 128:(mt + 1) * 128],
                                rhs=WihT_sb[:, k, n * 512:(n + 1) * 512],
                                start=(k == 0), stop=(k == 3))
                    nc.vector.tensor_add(xg[:, mt, :], ps[:], bias_bc[:])

            # ---- LSTM over 40 steps ----
            hiddensT = big.tile([128, 4, T, BC], BF, tag="hiddensT")
            c_prev = None
            lstm_ps = tc.tile_pool(name="psC", bufs=1, space="PSUM")
            htr_ps = tc.tile_pool(name="psCt", bufs=4, space="PSUM")
            gpsum, tpsum = lstm_ps.__enter__(), htr_ps.__enter__()
            for t in range(T):
                mt, po = (t * TB) // 128, (t * TB) % 128
                gps = gpsum.tile([BC, G4], F32, space="PSUM", tag="gates")
                for n in range(4):
                    ns = slice(n * 512, (n + 1) * 512)
                    if t > 0:
                        for k in range(4):
                            nc.tensor.matmul(
                                gps[:, ns],
                                lhsT=hiddensT[:, k, t - 1, :],
                                rhs=WhhT_sb[:, k, ns],
                                start=(k == 0), stop=False)
                    nc.tensor.matmul(
                        gps[:, ns],
                        lhsT=identb[po:po + BC, po:po + BC],
                        rhs=xg[po:po + BC, mt, ns],
                        start=(t == 0), stop=True,
                        tile_position=(po, 0))
                sig = state.tile([BC, 3 * H], F32, tag="sig")
                nc.scalar.activation(
                    sig[:], gps[:, 0:3 * H],
                    mybir.ActivationFunctionType.Sigmoid)
                g_t = state.tile([BC, H], F32, tag="g")
                nc.scalar.activation(
                    g_t[:], gps[:, 3 * H:4 * H],
                    mybir.ActivationFunctionType.Tanh)
                ig = state.tile([BC, H], F32, tag="ig")
                nc.vector.tensor_mul(ig[:], sig[:, 0:H], g_t[:])
                c_new = state.tile([BC, H], F32, tag="c")
                if t == 0:
                    nc.vector.tensor_copy(c_new[:], ig[:])
                else:
                    cf = state.tile([BC, H], F32, tag="cf")
                    nc.vector.tensor_mul(cf[:], sig[:, H:2 * H], c_prev[:])
                    nc.vector.tensor_add(c_new[:], cf[:], ig[:])
                c_prev = c_new
                tc_t = state.tile([BC, H], F32, tag="tanhc")
                nc.scalar.activation(
                    tc_t[:], c_new[:], mybir.ActivationFunctionType.Tanh)
                h_bf = state.tile([BC, H], BF, tag="h")
                nc.vector.tensor_mul(h_bf[:], sig[:, 2 * H:3 * H], tc_t[:])
                for e in range(4):
                    pst = tpsum.tile([128, BC], BF, space="PSUM", tag="htr")
                    nc.tensor.transpose(
                        pst[:], h_bf[:, e * 128:(e + 1) * 128],
                        identb[0:BC, 0:BC])
                    nc.scalar.copy(hiddensT[:, e, t, :], pst[:])
            htr_ps.__exit__(None, None, None)
            lstm_ps.__exit__(None, None, None)

            # ---- vocab GEMM: logitsT[v,640] = W_outT.T @ hiddensT + b_out ----
            with tc.tile_pool(name="psD", bufs=3, space="PSUM") as psD:
                for vt in range(NVT):
                    wt = work.tile([128, 4, 128], BF, tag="wout")
                    nc.sync.dma_start(
                        wt[:],
                        WoutT_d.ap().rearrange("(k p) v -> p k v", p=128)
                        [:, :, vt * 128:(vt + 1) * 128])
                    vps = psD.tile([128, NB], F32, space="PSUM", tag="vps")
                    for k in range(4):
                        nc.tensor.matmul(
                            vps[:, 0:512], lhsT=wt[:, k, :],
                            rhs=hiddensT[:, k, 0:32, :],
                            start=(k == 0), stop=(k == 3))
                        nc.tensor.matmul(
                            vps[:, 512:NB], lhsT=wt[:, k, :],
                            rhs=hiddensT[:, k, 32:T, :],
                            start=(k == 0), stop=(k == 3))
                    lsb = work.tile([128, NB], F32, tag="lout")
                    nc.scalar.activation(
                        lsb[:], vps[:], mybir.ActivationFunctionType.Identity,
                        bias=bout_sb[:, vt:vt + 1])
                    nc.sync.dma_start(
                        out_d.ap()[vt * 128:(vt + 1) * 128, :], lsb[:])

    nc.compile()
    _CACHE["nc"] = nc
    return nc


def kernel(features, seqs, lengths, W_in, b_in, emb, W_ih, W_hh, b_ih, b_hh,
           W_out, b_out):
    f32 = lambda x: np.asarray(x, dtype=np.float32)
    bf = lambda x: np.ascontiguousarray(f32(x)).astype(bfnp)
    features, seqs = f32(features), np.asarray(seqs).astype(np.int64)
    # gate order [i, f, o, g]
    perm = np.concatenate([np.arange(0, 2 * H), np.arange(3 * H, 4 * H),
                           np.arange(2 * H, 3 * H)])
    WinT = bf(f32(W_in).T)                     # [F, E]
    WihT = np.ascontiguousarray(bf(f32(W_ih).T)[:, perm])
    WhhT = np.ascontiguousarray(bf(f32(W_hh).T)[:, perm])
    bcomb = np.ascontiguousarray((f32(b_ih) + f32(b_hh))[perm])
    emb_b = bf(emb)
    WoutT = np.zeros((H, VP), dtype=bfnp)
    WoutT[:, :V] = bf(f32(W_out).T)
    boutp = np.zeros((VP,), np.float32)
    boutp[:V] = f32(b_out)
    binp = f32(b_in)

    nc = _build()
    in_maps = []
    for c in range(NCORES):
        bs = slice(c * BC, (c + 1) * BC)
        featT = bf(features[bs].T)             # [F, BC]
        idx = np.zeros((T, TB), np.int64)
        idx[1:, :BC] = seqs[bs].T              # t-major, t=0 block dummy
        in_maps.append({
            "featT": featT,
            "idx": idx.reshape(NTB, 1).astype(np.int32),
            "embt": emb_b,
            "WinT": WinT, "WihT": WihT, "WhhT": WhhT,
            "bcomb": bcomb, "bin": binp, "bout": boutp,
            "WoutT": WoutT,
        })
    _CACHE["last_in_maps"] = in_maps
    res = run_bass_kernel_spmd(nc, in_maps, list(range(NCORES)))
    out = np.empty((B, T, V), np.float32)
    for c in range(NCORES):
        lt = res.results[c]["out_lt"][:V]      # [V, 640]
        out[c * BC:(c + 1) * BC] = (
            lt.reshape(V, T, BC).transpose(2, 1, 0))
    return out


# revision 14
# speedup vs baseline: 1.1343x; 1.1343x over previous
"""Trainium2 Bass kernel for nn_Caption (LSTM caption decoder).

Distribution: pure data-parallel over batch (128 -> 8 cores x 16), no
collectives. Per core: x0 projection GEMM, embedding gather (device),
input-gate GEMM, 40-step LSTM recurrence, vocab GEMM [640,512]@[512,10000].

Layout strategy: all GEMM operands bf16 (fp32 PSUM accumulation); weights
host-transposed so the contraction dim lands on partitions; outputs
produced in T-layout (feature on partitions) so biases fuse into ACT
copies as per-partition bias. LSTM runs B-layout (batch on partitions)
with per-step h transposed via PE into hiddensT, which is consumed
directly by the vocab GEMM. xg is injected into the gates PSUM via
identity matmuls (t-blocks padded to 32 partitions for alignment).
"""
import sys

sys.path.insert(0, "/opt/trn_rl_repo")

import numpy as np
import ml_dtypes

import concourse.bass as bass
import concourse.tile as tile
from concourse import bacc, mybir
from concourse.bass_utils import run_bass_kernel_spmd
from concourse.masks import make_identity

BF = mybir.dt.bfloat16
F32 = mybir.dt.float32
I32 = mybir.dt.int32
bfnp = ml_dtypes.bfloat16

B, F, E, H, V, T = 128, 1536, 512, 512, 10000, 40
NCORES = 8
BC = B // NCORES          # 16 batch rows per core
TB = 32                   # padded t-block width (partition alignment)
NTB = T * TB              # 1280 padded (t,b) columns
NB = T * BC               # 640 real (t,b) columns
G4 = 4 * H                # 2048 gate dims, order [i, f, o, g]
VP = ((V + 127) // 128) * 128  # 10112 padded vocab
NVT = VP // 128           # 79 vocab tiles

_CACHE = {}


def _build():
    if "nc" in _CACHE:
        return _CACHE["nc"]
    nc = bacc.Bacc("TRN2", target_bir_lowering=False, debug=False,
                   num_devices=NCORES)

    featT_d = nc.dram_tensor("featT", [F, BC], BF, kind="ExternalInput")
    idx_d = nc.dram_tensor("idx", [NTB, 1], I32, kind="ExternalInput")
    emb_d = nc.dram_tensor("embt", [V, E], BF, kind="ExternalInput")
    WinT_d = nc.dram_tensor("WinT", [F, E], BF, kind="ExternalInput")
    WihT_d = nc.dram_tensor("WihT", [E, G4], BF, kind="ExternalInput")
    WhhT_d = nc.dram_tensor("WhhT", [H, G4], BF, kind="ExternalInput")
    bcomb_d = nc.dram_tensor("bcomb", [G4], F32, kind="ExternalInput")
    bin_d = nc.dram_tensor("bin", [E], F32, kind="ExternalInput")
    bout_d = nc.dram_tensor("bout", [VP], F32, kind="ExternalInput")
    WoutT_d = nc.dram_tensor("WoutT", [H, VP], BF, kind="ExternalInput")
    out_d = nc.dram_tensor("out_lt", [VP, NB], F32, kind="ExternalOutput")

    with tile.TileContext(nc) as tc:
        with (
            tc.tile_pool(name="consts", bufs=1) as consts,
            tc.tile_pool(name="big", bufs=1) as big,
            tc.tile_pool(name="state", bufs=2) as state,
            tc.tile_pool(name="work", bufs=3) as work,
            tc.tile_pool(name="wpool", bufs=4) as wpool,
        ):
            # ---- index load + constants ----
            idx_sb = consts.tile([128, 10, 1], I32)
            nc.sync.dma_start(
                idx_sb[:], idx_d.ap().rearrange("(j p) o -> p j o", p=128))
            identf = consts.tile([128, 128], F32)
            make_identity(nc, identf[:])
            identb = consts.tile([128, 128], BF)
            nc.vector.tensor_copy(identb[:], identf[:])

            WihT_sb = big.tile([128, 4, G4], BF, tag="wih")
            nc.sync.dma_start(
                WihT_sb[:], WihT_d.ap().rearrange("(k p) n -> p k n", p=128))
            WhhT_sb = big.tile([128, 4, G4], BF, tag="whh")
            nc.sync.dma_start(
                WhhT_sb[:], WhhT_d.ap().rearrange("(k p) n -> p k n", p=128))
            WinT_sb = big.tile([128, 12, E], BF, tag="win")
            nc.sync.dma_start(
                WinT_sb[:], WinT_d.ap().rearrange("(k p) n -> p k n", p=128))
            featT_sb = consts.tile([128, 12, BC], BF)
            nc.sync.dma_start(
                featT_sb[:], featT_d.ap().rearrange("(k p) b -> p k b", p=128))
            bias_bc = big.tile([128, G4], F32, tag="biasbc")
            nc.sync.dma_start(
                bias_bc[:],
                bass.AP(tensor=bcomb_d, offset=0, ap=[[0, 128], [1, G4]]))
            bin_sb = consts.tile([128, 4], F32)
            nc.sync.dma_start(
                bin_sb[:], bin_d.ap().rearrange("(k p) -> p k", p=128))
            bout_sb = consts.tile([128, NVT], F32)
            nc.sync.dma_start(
                bout_sb[:], bout_d.ap().rearrange("(k p) -> p k", p=128))

            # ---- embedding gather -> seqT (transposed via PE) ----
            seqT = big.tile([128, 4, NTB], BF, tag="seqT")
            with tc.tile_pool(name="psA", bufs=3, space="PSUM") as psA:
                for j in range(10):
                    gt = work.tile([128, E], BF, tag="gather")
                    nc.gpsimd.indirect_dma_start(
                        out=gt[:], out_offset=None, in_=emb_d.ap(),
                        in_offset=bass.IndirectOffsetOnAxis(
                            ap=idx_sb[:, j, :], axis=0))
                    for e in range(4):
                        pst = psA.tile([128, 128], BF, space="PSUM", tag="tr")
                        nc.tensor.transpose(
                            pst[:], gt[:, e * 128:(e + 1) * 128], identb[:])
                        nc.scalar.copy(
                            seqT[:, e, j * 128:(j + 1) * 128], pst[:])

                # ---- x0T = W_inT.T @ featT + b_in -> seqT[:, :, 0:BC] ----
                for m in range(4):
                    ps = psA.tile([128, BC], F32, space="PSUM", tag="x0")
                    for k in range(12):
                        nc.tensor.matmul(
                            ps[:], lhsT=WinT_sb[:, k, m * 128:(m + 1) * 128],
                            rhs=featT_sb[:, k, :],
                            start=(k == 0), stop=(k == 11))
                    nc.scalar.activation(
                        seqT[:, m, 0:BC], ps[:],
                        mybir.ActivationFunctionType.Identity,
                        bias=bin_sb[:, m:m + 1])

            # ---- LSTM + interleaved xg / vocab GEMM ----
            xg = big.tile([128, 10, G4], BF, tag="xg")
            hiddensT = big.tile([128, 4, T, BC], BF, tag="hiddensT")

            lstm_ps = tc.tile_pool(name="psGates", bufs=1, space="PSUM")
            htr_ps = tc.tile_pool(name="psHtr", bufs=2, space="PSUM")
            xg_ps = tc.tile_pool(name="psXg", bufs=2, space="PSUM")
            voc_ps = tc.tile_pool(name="psVoc", bufs=2, space="PSUM")
            gpsum = lstm_ps.__enter__()
            tpsum = htr_ps.__enter__()
            xgpsum = xg_ps.__enter__()
            vpsum = [None]

            def emit_xg_mtile(mt):
                for n in range(4):
                    ps = xgpsum.tile([128, 512], F32, space="PSUM", tag="xgps")
                    for k in range(4):
                        nc.tensor.matmul(
                            ps[:],
                            lhsT=seqT[:, k, mt * 128:(mt + 1) * 128],
                            rhs=WihT_sb[:, k, n * 512:(n + 1) * 512],
                            start=(k == 0), stop=(k == 3))
                    nc.vector.tensor_add(
                        xg[:, mt, n * 512:(n + 1) * 512], ps[:],
                        bias_bc[:, n * 512:(n + 1) * 512])

            # vocab windows over t: (0..15), (16..31), (32..39)
            VWIN = [(0, 16), (16, 32), (32, 40)]
            vunits = []          # ready (vt, w) units, appended as windows close
            ncopy = [0]

            def emit_vocab_unit(vt, w):
                t0, t1 = VWIN[w]
                nb = (t1 - t0) * BC
                wt = wpool.tile([128, 4, 128], BF, tag="wout")
                nc.sync.dma_start(
                    wt[:],
                    WoutT_d.ap().rearrange("(k p) v -> p k v", p=128)
                    [:, :, vt * 128:(vt + 1) * 128])
                vps = vpsum[0].tile([128, 256], F32, space="PSUM", tag="vps")
                for k in range(4):
                    nc.tensor.matmul(
                        vps[:, 0:nb], lhsT=wt[:, k, :],
                        rhs=hiddensT[:, k, t0:t1, :],
                        start=(k == 0), stop=(k == 3))
                lsb = work.tile([128, 256], F32, tag="lout")
                ncopy[0] += 1
                if ncopy[0] % 2 == 0:
                    nc.scalar.activation(
                        lsb[:, 0:nb], vps[:, 0:nb],
                        mybir.ActivationFunctionType.Identity,
                        bias=bout_sb[:, vt:vt + 1])
                else:
                    nc.vector.tensor_scalar_add(
                        out=lsb[:, 0:nb], in0=vps[:, 0:nb],
                        scalar1=bout_sb[:, vt:vt + 1])
                nc.sync.dma_start(
                    out_d.ap()[vt * 128:(vt + 1) * 128, t0 * BC:t1 * BC],
                    lsb[:, 0:nb])

            emit_xg_mtile(0)
            emit_xg_mtile(1)

            c_prev = None
            for t in range(T):
                if t == 16:
                    xg_ps.__exit__(None, None, None)
                    vpsum[0] = voc_ps.__enter__()
                mt, po = (t * TB) // 128, (t * TB) % 128
                gps = gpsum.tile([BC, G4], F32, space="PSUM", tag="gates")
                # order: g chunk (n=3) first, then i, f, o
                for n in (3, 0, 1, 2):
                    ns = slice(n * 512, (n + 1) * 512)
                    if t > 0:
                        for k in range(4):
                            nc.tensor.matmul(
                                gps[:, ns],
                                lhsT=hiddensT[:, k, t - 1, :],
                                rhs=WhhT_sb[:, k, ns],
                                start=(k == 0), stop=False)
                    nc.tensor.matmul(
                        gps[:, ns],
                        lhsT=identb[po:po + BC, po:po + BC],
                        rhs=xg[po:po + BC, mt, ns],
                        start=(t == 0), stop=True,
                        tile_position=(po, 0))
                    if n == 3:
                        g_t = state.tile([BC, H], F32, tag="g")
                        nc.scalar.activation(
                            g_t[:], gps[:, 3 * H:4 * H],
                            mybir.ActivationFunctionType.Tanh)
                    elif n == 1:
                        sig_if = state.tile([BC, 2 * H], F32, tag="sigif")
                        nc.scalar.activation(
                            sig_if[:], gps[:, 0:2 * H],
                            mybir.ActivationFunctionType.Sigmoid)
                sig_o = state.tile([BC, H], F32, tag="sigo")
                nc.scalar.activation(
                    sig_o[:], gps[:, 2 * H:3 * H],
                    mybir.ActivationFunctionType.Sigmoid)
                ig = state.tile([BC, H], F32, tag="ig")
                nc.vector.tensor_mul(ig[:], sig_if[:, 0:H], g_t[:])
                c_new = state.tile([BC, H], F32, tag="c")
                if t == 0:
                    nc.vector.tensor_copy(c_new[:], ig[:])
                else:
                    cf = state.tile([BC, H], F32, tag="cf")
                    nc.vector.tensor_mul(cf[:], sig_if[:, H:2 * H], c_prev[:])
                    nc.vector.tensor_add(c_new[:], cf[:], ig[:])
                c_prev = c_new
                tc_t = state.tile([BC, H], F32, tag="tanhc")
                nc.scalar.activation(
                    tc_t[:], c_new[:], mybir.ActivationFunctionType.Tanh)
                h_bf = state.tile([BC, H], BF, tag="h")
                nc.vector.tensor_mul(h_bf[:], sig_o[:], tc_t[:])
                pst = tpsum.tile([128, 4 * BC], BF, space="PSUM", tag="htr")
                for e in range(4):
                    nc.tensor.transpose(
                        pst[:, e * BC:(e + 1) * BC],
                        h_bf[:, e * 128:(e + 1) * 128],
                        identb[0:BC, 0:BC])
                nc.scalar.copy(hiddensT[:, :, t, :], pst[:])

                # interleaved filler work
                if t < 16 and t % 2 == 0 and t // 2 + 2 < 10:
                    emit_xg_mtile(t // 2 + 2)
                for w, (t0, t1) in enumerate(VWIN):
                    if t == t1 - 1:
                        vunits.extend((vt, w) for vt in range(NVT))
                if t >= 16:
                    for _ in range(3):
                        if vunits:
                            emit_vocab_unit(*vunits.pop(0))

            # vocab tail
            while vunits:
                emit_vocab_unit(*vunits.pop(0))

            voc_ps.__exit__(None, None, None)
            htr_ps.__exit__(None, None, None)
            lstm_ps.__exit__(None, None, None)

    nc.compile()
    _CACHE["nc"] = nc
    return nc


def kernel(features, seqs, lengths, W_in, b_in, emb, W_ih, W_hh, b_ih, b_hh,
           W_out, b_out):
    f32 = lambda x: np.asarray(x, dtype=np.float32)
    bf = lambda x: np.ascontiguousarray(f32(x)).astype(bfnp)
    features, seqs = f32(features), np.asarray(seqs).astype(np.int64)
    # gate order [i, f, o, g]
    perm = np.concatenate([np.arange(0, 2 * H), np.arange(3 * H, 4 * H),
                           np.arange(2 * H, 3 * H)])
    WinT = bf(f32(W_in).T)                     # [F, E]
    WihT = np.ascontiguousarray(bf(f32(W_ih).T)[:, perm])
    WhhT = np.ascontiguousarray(bf(f32(W_hh).T)[:, perm])
    bcomb = np.ascontiguousarray((f32(b_ih) + f32(b_hh))[perm])
    emb_b = bf(emb)
    WoutT = np.zeros((H, VP), dtype=bfnp)
    WoutT[:, :V] = bf(f32(W_out).T)
    boutp = np.zeros((VP,), np.float32)
    boutp[:V] = f32(b_out)
    binp = f32(b_in)

    nc = _build()
    in_maps = []
    for c in range(NCORES):
        bs = slice(c * BC, (c + 1) * BC)
        featT = bf(features[bs].T)             # [F, BC]
        idx = np.zeros((T, TB), np.int64)
        idx[1:, :BC] = seqs[bs].T              # t-major, t=0 block dummy
        in_maps.append({
            "featT": featT,
            "idx": idx.reshape(NTB, 1).astype(np.int32),
            "embt": emb_b,
            "WinT": WinT, "WihT": WihT, "WhhT": WhhT,
            "bcomb": bcomb, "bin": binp, "bout": boutp,
            "WoutT": WoutT,
        })
    _CACHE["last_in_maps"] = in_maps
    res = run_bass_kernel_spmd(nc, in_maps, list(range(NCORES)))
    out = np.empty((B, T, V), np.float32)
    for c in range(NCORES):
        lt = res.results[c]["out_lt"][:V]      # [V, 640]
        out[c * BC:(c + 1) * BC] = (
            lt.reshape(V, T, BC).transpose(2, 1, 0))
    return out
